# revision 43
# baseline (speedup 1.0000x reference)
"""Chunked cross-attention TRN2 kernel (8 NeuronCores, SPMD).

Problem (hardcoded): B=4, S=2048, HIDDEN=1024, heads=16, head_dim=64,
chunks C=32 x chunk_len 64, neighbors N=2 x L=128 (256 keys per chunk).

Sharding: the B*C = 128 (batch, chunk) pairs are split evenly across the 8
cores (16 pairs each). Each core projects Q/K/V for its pairs, runs the
chunk-local softmax attention, and writes its pairs' outputs. Weights are
replicated per core. No collectives needed.

All matmul operands are bf16 (stationary tiles are 128-column so FWL is
active; fp32 accumulation in PSUM). The attention avoids PE transposes
entirely: scores are computed pre-transposed as out[keys, queries] with a
block-diagonal [dA|dB]x[qA|qB] moving operand built during the Q
projection (zeros kill the cross-head terms), packing two heads per
matmul. The AV matmul consumes the [keys, q] attention weights directly;
a ones-column appended to V yields the softmax normalizer in the same
matmul, and the 1/sum scale is applied by ScalarE during PSUM copyback.

Per-block schedule interleaves the V projection with attention per pair
so the ScalarE exp runs under PE matmuls of the next pair.
"""

import numpy as np

B, S, HID = 4, 2048, 1024
C, NNB, L = 32, 2, 128
CHUNK = 64
NHEADS, HEAD = 16, 64
NCORES = 8
NPAIRS = B * C                 # 128
PER_CORE = NPAIRS // NCORES    # 16
QBLK = 4                       # pairs per block (256 q rows, 1024 kv rows)
NBLK = PER_CORE // QBLK        # 4
P = 128
KSUB = HID // P                # 8
MSUB = HID // P                # 8
JTOT = NNB * L                 # 256 keys per chunk
SCALE = 0.125                  # 1/sqrt(HEAD)

_CACHE = {}


def _build():
    from contextlib import ExitStack

    import concourse.bass as bass
    import concourse.mybir as mybir
    import concourse.tile as tile
    from concourse import bacc

    f32 = mybir.dt.float32
    bf16 = mybir.dt.bfloat16

    nc = bacc.Bacc("TRN2", target_bir_lowering=False, debug=False,
                   num_devices=NCORES)

    ROWS = QBLK * JTOT             # 1024 kv rows per block
    GRP = ROWS // 2                # 512 rows per K-projection group
    QR = QBLK * CHUNK              # 256 q rows per block

    # all inputs are host-packed partition-major so every DMA is 128
    # large contiguous segments (cheap DIRECT2D descriptor generation)
    q_t = nc.dram_tensor("q_t", [NBLK // 2, P, KSUB, 2 * QR], bf16,
                         kind="ExternalInput")
    kv_t = nc.dram_tensor("kv_t", [2 * NBLK, P, KSUB, GRP], bf16,
                          kind="ExternalInput")
    wq_t = nc.dram_tensor("wq_t", [P, MSUB, KSUB, P], bf16, kind="ExternalInput")
    wk_t = nc.dram_tensor("wk_t", [P, MSUB, KSUB, P], bf16, kind="ExternalInput")
    wv_t = nc.dram_tensor("wv_t", [P, KSUB, HID], bf16, kind="ExternalInput")
    bqk_t = nc.dram_tensor("bqk_t", [P, 2 * MSUB], f32, kind="ExternalInput")
    bv_r = nc.dram_tensor("bv_r", [P, HID], f32, kind="ExternalInput")
    out = nc.dram_tensor("out", [PER_CORE, P, NHEADS * HEAD // 2], f32,
                         kind="ExternalOutput")

    with tile.TileContext(nc) as tc:
        with ExitStack() as ctx:
            wpool = ctx.enter_context(tc.tile_pool(name="weights", bufs=1))
            qtp = ctx.enter_context(tc.tile_pool(name="qt", bufs=2))
            kvp = ctx.enter_context(tc.tile_pool(name="kvt", bufs=3))
            kpp = ctx.enter_context(tc.tile_pool(name="kproj", bufs=1))
            sfp = ctx.enter_context(tc.tile_pool(name="soft", bufs=24))
            smalls = ctx.enter_context(tc.tile_pool(name="smalls", bufs=6))
            outp = ctx.enter_context(tc.tile_pool(name="outsb", bufs=2))
            ps_pj = ctx.enter_context(tc.tile_pool(name="ps_pj", bufs=3, space="PSUM"))
            ps_sc = ctx.enter_context(tc.tile_pool(name="ps_sc", bufs=2, space="PSUM"))
            ps_av = ctx.enter_context(tc.tile_pool(name="ps_av", bufs=3, space="PSUM"))

            # persistent ping-pong tiles: block-diagonal q (off-diagonal
            # zeros written once here, diagonal rewritten per block) and
            # V-with-ones-column (col 128 written once).
            qpbd_ts = []
            vp_ts = []
            for i in range(2):
                qpbd = wpool.tile([P, 2 * QBLK, MSUB, P], bf16,
                                  name=f"qpbd{i}")
                # only the off-diagonal blocks need zeros; the diagonal
                # is rewritten by every Q projection
                nc.vector.memset(qpbd[0:64, :, :, 64:128], 0.0)
                nc.vector.memset(qpbd[64:128, :, :, 0:64], 0.0)
                qpbd_ts.append(qpbd)
                vp = wpool.tile([P, 2 * QBLK, MSUB, P + 1], bf16,
                                name=f"vp{i}")
                nc.vector.memset(vp[:, :, :, P], 1.0)
                vp_ts.append(vp)

            Exp = mybir.ActivationFunctionType.Exp
            Ident = mybir.ActivationFunctionType.Identity

            # --- resident constants; one dma_start each (descriptor
            # generation costs ~0.7us serial on the sync queue) ---
            # wq m-tile 0 and qt k-chunks split out so the first Q chain
            # starts after ~0.4MB and streams as its inputs trickle in
            wq_sb = wpool.tile([P, MSUB, KSUB, P], bf16)
            nc.sync.dma_start(wq_sb[:, 0], wq_t[:, 0])
            qt_ts = []
            for sb in range(2):
                qt_sb = qtp.tile([P, KSUB, 2 * QR], bf16, tag="qt",
                                 name="qt_sb")
                if sb == 0:
                    for k in range(KSUB):
                        nc.sync.dma_start(qt_sb[:, k, :], q_t[0, :, k, :])
                    nc.sync.dma_start(wq_sb[:, 1:MSUB], wq_t[:, 1:MSUB])
                else:
                    nc.sync.dma_start(qt_sb[:], q_t[sb])
                qt_ts.append(qt_sb)
            bqk_sb = wpool.tile([P, 2 * MSUB], f32)
            nc.sync.dma_start(bqk_sb[:], bqk_t[:])
            wk_sb = wpool.tile([P, MSUB, KSUB, P], bf16)
            nc.sync.dma_start(wk_sb[:], wk_t[:])
            kvt_first = []
            for g2 in range(2):
                kvt_sb = kvp.tile([P, KSUB, GRP], bf16, tag="kvt",
                                  name="kvt_sb")
                nc.sync.dma_start(kvt_sb[:], kv_t[g2])
                kvt_first.append(kvt_sb)
            wv_sb = wpool.tile([P, KSUB, HID], bf16)
            nc.sync.dma_start(wv_sb[:], wv_t[:])
            bv_sb = wpool.tile([P, HID], f32)
            nc.sync.dma_start(bv_sb[:], bv_r[:])

            # ---- the whole Q projection runs upfront (28us of PE work
            # covering the weight/kv transfer window); each qpbd tile is
            # written exactly once ----
            for sb in range(2):
                qpbd = qpbd_ts[sb]
                for mo in range(MSUB):
                    pt = ps_pj.tile([P, 512], f32, tag="ps_pj", name="pt")
                    for k in range(KSUB):
                        nc.tensor.matmul(
                            pt[:],
                            wq_sb[:, mo, k, :],
                            qt_ts[sb][:, k, :],
                            start=(k == 0),
                            stop=(k == KSUB - 1),
                        )
                    nc.scalar.activation(
                        qpbd[0:64, :, mo, 0:64], pt[0:64, :], Ident,
                        bias=bqk_sb[0:64, mo, None])
                    nc.scalar.activation(
                        qpbd[64:128, :, mo, 64:128], pt[64:128, :], Ident,
                        bias=bqk_sb[64:128, mo, None])

            for blk in range(NBLK):
                qpbd = qpbd_ts[blk // 2]
                vp = vp_ts[blk % 2]
                po = (blk % 2) * QBLK   # this block's pair offset in qpbd

                if blk == 0:
                    kvt_ts = kvt_first
                else:
                    kvt_ts = []
                    for g2 in range(2):
                        kvt_sb = kvp.tile([P, KSUB, GRP], bf16, tag="kvt",
                                          name="kvt_sb")
                        nc.sync.dma_start(kvt_sb[:], kv_t[2 * blk + g2])
                        kvt_ts.append(kvt_sb)

                # ---- K projection (2 groups of 512 kv rows) ----
                kp_sb = kpp.tile([P, MSUB, ROWS], bf16, tag="kp", name="kp")
                for g2 in range(2):
                    for mo in range(MSUB):
                        pt = ps_pj.tile([P, 512], f32, tag="ps_pj", name="pt")
                        for k in range(KSUB):
                            nc.tensor.matmul(
                                pt[:],
                                wk_sb[:, mo, k, :],
                                kvt_ts[g2][:, k, :],
                                start=(k == 0),
                                stop=(k == KSUB - 1),
                            )
                        nc.scalar.activation(
                            kp_sb[:, mo, bass.ts(g2, GRP)], pt[:],
                            Ident, bias=bqk_sb[:, MSUB + mo, None])

                # ---- V projection for one pair (512 rows = 2 row-tiles) ----
                def v_proj(pi):
                    for rt in (2 * pi, 2 * pi + 1):
                        g2, rl = rt // 4, rt % 4
                        for nt in range(2):
                            pt = ps_pj.tile([P, 512], f32, tag="ps_pj",
                                            name="pt")
                            for k in range(KSUB):
                                nc.tensor.matmul(
                                    pt[:],
                                    kvt_ts[g2][:, k, bass.ts(rl, P)],
                                    wv_sb[:, k, bass.ts(nt, 512)],
                                    start=(k == 0),
                                    stop=(k == KSUB - 1),
                                )
                            nc.vector.tensor_tensor(
                                vp[:, rt, 4 * nt:4 * nt + 4, 0:P],
                                pt[:],
                                bv_sb[:, bass.ts(nt, 512)],
                                mybir.AluOpType.add,
                            )

                # ---- scores (pre-transposed, 2 heads per matmul) ----
                def scores(pi):
                    ps_list = []
                    for hp in range(MSUB):
                        ps_s = ps_sc.tile([P, 2, P], f32, tag="ps_s")
                        for jh in range(2):
                            nc.tensor.matmul(
                                ps_s[:, jh, :],
                                kp_sb[:, hp, bass.ds(pi * JTOT + jh * P, P)],
                                qpbd[:, po + pi, hp, :],
                                start=True, stop=True,
                            )
                        attn = sfp.tile([P, 2, P], bf16, tag="attn")
                        nc.scalar.activation(attn[:], ps_s[:], Exp,
                                             scale=SCALE)
                        ps_list.append(attn)
                    return ps_list

                # ---- AV + normalize + store for one pair ----
                # (vp/base bound at def time: the last pair's call is
                # deferred into the next block iteration)
                def att_out(pi, attn_ts, vp=vp, base=blk * QBLK):
                    out_sb = outp.tile([P, MSUB, HEAD], f32, tag="out_sb")
                    for hp in range(MSUB):
                        attn = attn_ts[hp]
                        ps_o = ps_av.tile([P, P + 1], f32, tag="ps_o")
                        for jh in range(2):
                            nc.tensor.matmul(
                                ps_o[:],
                                attn[:, jh, :],
                                vp[:, 2 * pi + jh, hp, :],
                                start=(jh == 0), stop=(jh == 1),
                            )
                        recip = smalls.tile([P, 1], f32, tag="recip")
                        nc.vector.reciprocal(recip[:], ps_o[:, P, None])
                        # normalize split across ScalarE/VectorE to keep
                        # both under the PE time of the attention phase
                        nc.scalar.activation(
                            out_sb[0:64, hp, :], ps_o[0:64, 0:64],
                            Ident, scale=recip[0:64])
                        nc.vector.tensor_scalar_mul(
                            out_sb[64:128, hp, :], ps_o[64:128, 64:128],
                            recip[64:128])
                    nc.sync.dma_start(out[base + pi], out_sb[:])

                # finish the previous block's last pair now that this
                # block's Q/K projections have covered its exp latency
                if blk > 0:
                    prev_att_out(QBLK - 1, prev_attn)

                # interleave: S(p) precedes V(p) so the exp chain hides
                # under the V projection; the last pair's AV is deferred
                # into the next block
                attn_p = scores(0)
                v_proj(0)
                attn_n = scores(1)
                v_proj(1)
                for pi in range(2, QBLK):
                    att_out(pi - 2, attn_p)
                    attn_p, attn_n = attn_n, scores(pi)
                    v_proj(pi)
                att_out(QBLK - 2, attn_p)
                prev_att_out = att_out
                prev_attn = attn_n

            prev_att_out(QBLK - 1, prev_attn)

    nc.finalize()
    return nc


def _prepare_inputs(query, kv, Wq, bq, Wk, bk, Wv, bv):
    """Build the 8 per-core input maps (host-side shard + layout + cast)."""
    import ml_dtypes

    f32 = np.float32
    bf = ml_dtypes.bfloat16
    query = np.asarray(query, dtype=f32)
    kv = np.asarray(kv, dtype=f32)

    # shift right by CHUNK-1, pad to C*CHUNK rows
    q_shift = np.zeros((B, C * CHUNK, HID), dtype=f32)
    q_shift[:, : S - (CHUNK - 1)] = query[:, CHUNK - 1:]
    q_pairs = q_shift.reshape(B * C, CHUNK, HID)
    kv_pairs = kv.reshape(B * C, JTOT, HID)

    QR = QBLK * CHUNK
    GRP = QBLK * JTOT // 2
    NGRP = PER_CORE * JTOT // GRP

    # partition-major packings: [.., p, ko, cols]
    wq_tt = np.asarray(Wq, dtype=f32).T  # [h, m]
    wq_t = np.ascontiguousarray(
        wq_tt.reshape(KSUB, P, MSUB, P).transpose(1, 2, 0, 3).astype(bf))
    wk_tt = np.asarray(Wk, dtype=f32).T  # [h, m]
    wk_t = np.ascontiguousarray(
        wk_tt.reshape(KSUB, P, MSUB, P).transpose(1, 2, 0, 3).astype(bf))
    wv_t = np.ascontiguousarray(
        np.asarray(Wv, dtype=f32).T.reshape(KSUB, P, HID)
        .transpose(1, 0, 2).astype(bf))
    bqk_t = np.ascontiguousarray(np.concatenate([
        np.asarray(bq, dtype=f32).reshape(MSUB, P).T,
        np.asarray(bk, dtype=f32).reshape(MSUB, P).T], axis=1))
    bv_rep = np.ascontiguousarray(
        np.broadcast_to(np.asarray(bv, dtype=f32), (P, HID)))

    in_maps = []
    for ci in range(NCORES):
        sel = slice(ci * PER_CORE, (ci + 1) * PER_CORE)
        q_core = q_pairs[sel].reshape(PER_CORE * CHUNK, HID)
        kv_core = kv_pairs[sel].reshape(PER_CORE * JTOT, HID)
        q_tc = np.ascontiguousarray(
            q_core.T.reshape(KSUB, P, NBLK // 2, 2 * QR)
            .transpose(2, 1, 0, 3).astype(bf))
        kv_tc = np.ascontiguousarray(
            kv_core.T.reshape(KSUB, P, NGRP, GRP)
            .transpose(2, 1, 0, 3).astype(bf))
        in_maps.append({
            "q_t": q_tc,
            "kv_t": kv_tc,
            "wq_t": wq_t,
            "wk_t": wk_t,
            "wv_t": wv_t,
            "bqk_t": bqk_t,
            "bv_r": bv_rep,
        })
    return in_maps


def _unpack_output(results):
    """results: list of 8 dicts with 'out' [16, 128, 512] -> full (B,S,HID)."""
    h = np.empty((NPAIRS, CHUNK, HID), dtype=np.float32)
    for ci in range(NCORES):
        arr = results[ci]["out"]
        a = arr.reshape(PER_CORE, 2, CHUNK, NHEADS // 2, HEAD)
        a = a.transpose(0, 2, 3, 1, 4).reshape(PER_CORE, CHUNK, HID)
        h[ci * PER_CORE:(ci + 1) * PER_CORE] = a
    h = h.reshape(B, C * CHUNK, HID)
    outp = np.zeros((B, S, HID), dtype=np.float32)
    outp[:, CHUNK - 1:] = h[:, : S - (CHUNK - 1)]
    return outp


def kernel(query, kv, Wq, bq, Wk, bk, Wv, bv):
    from concourse.bass_utils import run_bass_kernel_spmd

    if "nc" not in _CACHE:
        _CACHE["nc"] = _build()
    nc = _CACHE["nc"]

    in_maps = _prepare_inputs(query, kv, Wq, bq, Wk, bk, Wv, bv)
    res = run_bass_kernel_spmd(nc, in_maps, list(range(NCORES)))
    return _unpack_output(res.results)


# revision 51
# speedup vs baseline: 1.0120x; 1.0120x over previous
"""Chunked cross-attention TRN2 kernel (8 NeuronCores, SPMD).

Problem (hardcoded): B=4, S=2048, HIDDEN=1024, heads=16, head_dim=64,
chunks C=32 x chunk_len 64, neighbors N=2 x L=128 (256 keys per chunk).

Sharding: the B*C = 128 (batch, chunk) pairs are split evenly across the 8
cores (16 pairs each). Each core projects Q/K/V for its pairs, runs the
chunk-local softmax attention, and writes its pairs' outputs. Weights are
replicated per core. No collectives needed.

All matmul operands are bf16 (stationary tiles are 128-column so FWL is
active; fp32 accumulation in PSUM). The attention avoids PE transposes
entirely: scores are computed pre-transposed as out[keys, queries] with a
block-diagonal [dA|dB]x[qA|qB] moving operand built during the Q
projection (zeros kill the cross-head terms), packing two heads per
matmul. The AV matmul consumes the [keys, q] attention weights directly;
a ones-column appended to V yields the softmax normalizer in the same
matmul, and the 1/sum scale is applied by ScalarE during PSUM copyback.

Per-block schedule interleaves the V projection with attention per pair
so the ScalarE exp runs under PE matmuls of the next pair.
"""

import numpy as np

B, S, HID = 4, 2048, 1024
C, NNB, L = 32, 2, 128
CHUNK = 64
NHEADS, HEAD = 16, 64
NCORES = 8
NPAIRS = B * C                 # 128
PER_CORE = NPAIRS // NCORES    # 16
QBLK = 4                       # pairs per block (256 q rows, 1024 kv rows)
NBLK = PER_CORE // QBLK        # 4
P = 128
KSUB = HID // P                # 8
MSUB = HID // P                # 8
JTOT = NNB * L                 # 256 keys per chunk
SCALE = 0.125                  # 1/sqrt(HEAD)

_CACHE = {}


def _build():
    from contextlib import ExitStack

    import concourse.bass as bass
    import concourse.mybir as mybir
    import concourse.tile as tile
    from concourse import bacc

    f32 = mybir.dt.float32
    bf16 = mybir.dt.bfloat16

    nc = bacc.Bacc("TRN2", target_bir_lowering=False, debug=False,
                   num_devices=NCORES)

    ROWS = QBLK * JTOT             # 1024 kv rows per block
    GRP = ROWS // 2                # 512 rows per K-projection group
    QR = QBLK * CHUNK              # 256 q rows per block

    # all inputs are host-packed partition-major so every DMA is 128
    # large contiguous segments (cheap DIRECT2D descriptor generation)
    q_t = nc.dram_tensor("q_t", [NBLK // 2, P, KSUB, 2 * QR], bf16,
                         kind="ExternalInput")
    kv_t = nc.dram_tensor("kv_t", [2 * NBLK, P, KSUB, GRP], bf16,
                          kind="ExternalInput")
    wq_t = nc.dram_tensor("wq_t", [P, MSUB, KSUB, P], bf16, kind="ExternalInput")
    wk_t = nc.dram_tensor("wk_t", [P, MSUB, KSUB, P], bf16, kind="ExternalInput")
    wv_t = nc.dram_tensor("wv_t", [P, KSUB, HID], bf16, kind="ExternalInput")
    bqk_t = nc.dram_tensor("bqk_t", [P, 2 * MSUB], f32, kind="ExternalInput")
    out = nc.dram_tensor("out", [PER_CORE, P, NHEADS * HEAD // 2], f32,
                         kind="ExternalOutput")

    with tile.TileContext(nc) as tc:
        with ExitStack() as ctx:
            wpool = ctx.enter_context(tc.tile_pool(name="weights", bufs=1))
            qtp = ctx.enter_context(tc.tile_pool(name="qt", bufs=2))
            kvp = ctx.enter_context(tc.tile_pool(name="kvt", bufs=3))
            kpp = ctx.enter_context(tc.tile_pool(name="kproj", bufs=1))
            sfp = ctx.enter_context(tc.tile_pool(name="soft", bufs=24))
            smalls = ctx.enter_context(tc.tile_pool(name="smalls", bufs=6))
            outp = ctx.enter_context(tc.tile_pool(name="outsb", bufs=2))
            ps_pj = ctx.enter_context(tc.tile_pool(name="ps_pj", bufs=3, space="PSUM"))
            ps_sc = ctx.enter_context(tc.tile_pool(name="ps_sc", bufs=2, space="PSUM"))
            ps_av = ctx.enter_context(tc.tile_pool(name="ps_av", bufs=3, space="PSUM"))

            # persistent ping-pong tiles: block-diagonal q (off-diagonal
            # zeros written once here, diagonal rewritten per block) and
            # V-with-ones-column (col 128 written once).
            qpbd_ts = []
            vp_ts = []
            for i in range(2):
                qpbd = wpool.tile([P, 2 * QBLK, MSUB, P], bf16,
                                  name=f"qpbd{i}")
                # only the off-diagonal blocks need zeros; the diagonal
                # is rewritten by every Q projection
                nc.vector.memset(qpbd[0:64, :, :, 64:128], 0.0)
                nc.vector.memset(qpbd[64:128, :, :, 0:64], 0.0)
                qpbd_ts.append(qpbd)
                vp = wpool.tile([P, 2 * QBLK, MSUB, P + 1], bf16,
                                name=f"vp{i}")
                nc.vector.memset(vp[:, :, :, P], 1.0)
                vp_ts.append(vp)

            Exp = mybir.ActivationFunctionType.Exp
            Ident = mybir.ActivationFunctionType.Identity

            # --- resident constants; one dma_start each (descriptor
            # generation costs ~0.7us serial on the sync queue) ---
            # wq m-tile 0 and qt k-chunks split out so the first Q chain
            # starts after ~0.4MB and streams as its inputs trickle in
            wq_sb = wpool.tile([P, MSUB, KSUB, P], bf16)
            nc.sync.dma_start(wq_sb[:, 0], wq_t[:, 0])
            qt_ts = []
            for sb in range(2):
                qt_sb = qtp.tile([P, KSUB, 2 * QR], bf16, tag="qt",
                                 name="qt_sb")
                if sb == 0:
                    for k in range(KSUB):
                        nc.sync.dma_start(qt_sb[:, k, :], q_t[0, :, k, :])
                    nc.sync.dma_start(wq_sb[:, 1:MSUB], wq_t[:, 1:MSUB])
                else:
                    nc.sync.dma_start(qt_sb[:], q_t[sb])
                qt_ts.append(qt_sb)
            bqk_sb = wpool.tile([P, 2 * MSUB], f32)
            nc.sync.dma_start(bqk_sb[:], bqk_t[:])
            wk_sb = wpool.tile([P, MSUB, KSUB, P], bf16)
            nc.sync.dma_start(wk_sb[:], wk_t[:])
            kvt_first = []
            for g2 in range(2):
                kvt_sb = kvp.tile([P, KSUB, GRP], bf16, tag="kvt",
                                  name="kvt_sb")
                nc.sync.dma_start(kvt_sb[:], kv_t[g2])
                kvt_first.append(kvt_sb)
            wv_sb = wpool.tile([P, KSUB, HID], bf16)
            nc.sync.dma_start(wv_sb[:], wv_t[:])

            # ---- the whole Q projection runs upfront (28us of PE work
            # covering the weight/kv transfer window); each qpbd tile is
            # written exactly once ----
            for sb in range(2):
                qpbd = qpbd_ts[sb]
                for mo in range(MSUB):
                    pt = ps_pj.tile([P, 512], f32, tag="ps_pj", name="pt")
                    for k in range(KSUB):
                        nc.tensor.matmul(
                            pt[:],
                            wq_sb[:, mo, k, :],
                            qt_ts[sb][:, k, :],
                            start=(k == 0),
                            stop=(k == KSUB - 1),
                        )
                    nc.scalar.activation(
                        qpbd[0:64, :, mo, 0:64], pt[0:64, :], Ident,
                        bias=bqk_sb[0:64, mo, None])
                    nc.scalar.activation(
                        qpbd[64:128, :, mo, 64:128], pt[64:128, :], Ident,
                        bias=bqk_sb[64:128, mo, None])

            for blk in range(NBLK):
                qpbd = qpbd_ts[blk // 2]
                vp = vp_ts[blk % 2]
                po = (blk % 2) * QBLK   # this block's pair offset in qpbd

                if blk == 0:
                    kvt_ts = kvt_first
                else:
                    kvt_ts = []
                    for g2 in range(2):
                        kvt_sb = kvp.tile([P, KSUB, GRP], bf16, tag="kvt",
                                          name="kvt_sb")
                        nc.sync.dma_start(kvt_sb[:], kv_t[2 * blk + g2])
                        kvt_ts.append(kvt_sb)

                # ---- K projection (2 groups of 512 kv rows) ----
                kp_sb = kpp.tile([P, MSUB, ROWS], bf16, tag="kp", name="kp")
                for g2 in range(2):
                    for mo in range(MSUB):
                        pt = ps_pj.tile([P, 512], f32, tag="ps_pj", name="pt")
                        for k in range(KSUB):
                            nc.tensor.matmul(
                                pt[:],
                                wk_sb[:, mo, k, :],
                                kvt_ts[g2][:, k, :],
                                start=(k == 0),
                                stop=(k == KSUB - 1),
                            )
                        nc.scalar.activation(
                            kp_sb[:, mo, bass.ts(g2, GRP)], pt[:],
                            Ident, bias=bqk_sb[:, MSUB + mo, None])

                # ---- V projection for one pair (512 rows = 2 row-tiles) ----
                def v_proj(pi):
                    for rt in (2 * pi, 2 * pi + 1):
                        g2, rl = rt // 4, rt % 4
                        for nt in range(2):
                            pt = ps_pj.tile([P, 512], f32, tag="ps_pj",
                                            name="pt")
                            for k in range(KSUB):
                                nc.tensor.matmul(
                                    pt[:],
                                    kvt_ts[g2][:, k, bass.ts(rl, P)],
                                    wv_sb[:, k, bass.ts(nt, 512)],
                                    start=(k == 0),
                                    stop=(k == KSUB - 1),
                                )
                            # V bias is added on the host (softmax weights
                            # sum to 1, so +bv commutes past the attention)
                            nc.vector.tensor_copy(
                                vp[:, rt, 4 * nt:4 * nt + 4, 0:P], pt[:])

                # ---- scores (pre-transposed, 2 heads per matmul) ----
                def scores(pi):
                    ps_list = []
                    for hp in range(MSUB):
                        ps_s = ps_sc.tile([P, 2, P], f32, tag="ps_s")
                        for jh in range(2):
                            nc.tensor.matmul(
                                ps_s[:, jh, :],
                                kp_sb[:, hp, bass.ds(pi * JTOT + jh * P, P)],
                                qpbd[:, po + pi, hp, :],
                                start=True, stop=True,
                            )
                        attn = sfp.tile([P, 2, P], bf16, tag="attn")
                        nc.scalar.activation(attn[:], ps_s[:], Exp,
                                             scale=SCALE)
                        ps_list.append(attn)
                    return ps_list

                # ---- AV + normalize + store for one pair ----
                # (vp/base bound at def time: the last pair's call is
                # deferred into the next block iteration)
                def att_out(pi, attn_ts, vp=vp, base=blk * QBLK):
                    out_sb = outp.tile([P, MSUB, HEAD], f32, tag="out_sb")
                    for hp in range(MSUB):
                        attn = attn_ts[hp]
                        ps_o = ps_av.tile([P, P + 1], f32, tag="ps_o")
                        for jh in range(2):
                            nc.tensor.matmul(
                                ps_o[:],
                                attn[:, jh, :],
                                vp[:, 2 * pi + jh, hp, :],
                                start=(jh == 0), stop=(jh == 1),
                            )
                        recip = smalls.tile([P, 1], f32, tag="recip")
                        nc.vector.reciprocal(recip[:], ps_o[:, P, None])
                        # normalize split across ScalarE/VectorE to keep
                        # both under the PE time of the attention phase
                        nc.scalar.activation(
                            out_sb[0:64, hp, :], ps_o[0:64, 0:64],
                            Ident, scale=recip[0:64])
                        nc.vector.tensor_scalar_mul(
                            out_sb[64:128, hp, :], ps_o[64:128, 64:128],
                            recip[64:128])
                    nc.sync.dma_start(out[base + pi], out_sb[:])

                # finish the previous block's last pair now that this
                # block's Q/K projections have covered its exp latency
                if blk > 0:
                    prev_att_out(QBLK - 1, prev_attn)

                # interleave: S(p) precedes V(p) so the exp chain hides
                # under the V projection; the last pair's AV is deferred
                # into the next block
                attn_p = scores(0)
                v_proj(0)
                attn_n = scores(1)
                v_proj(1)
                for pi in range(2, QBLK):
                    att_out(pi - 2, attn_p)
                    attn_p, attn_n = attn_n, scores(pi)
                    v_proj(pi)
                att_out(QBLK - 2, attn_p)
                prev_att_out = att_out
                prev_attn = attn_n

            prev_att_out(QBLK - 1, prev_attn)

    nc.finalize()
    return nc


def _prepare_inputs(query, kv, Wq, bq, Wk, bk, Wv, bv):
    """Build the 8 per-core input maps (host-side shard + layout + cast)."""
    import ml_dtypes

    f32 = np.float32
    bf = ml_dtypes.bfloat16
    query = np.asarray(query, dtype=f32)
    kv = np.asarray(kv, dtype=f32)

    # shift right by CHUNK-1, pad to C*CHUNK rows
    q_shift = np.zeros((B, C * CHUNK, HID), dtype=f32)
    q_shift[:, : S - (CHUNK - 1)] = query[:, CHUNK - 1:]
    q_pairs = q_shift.reshape(B * C, CHUNK, HID)
    kv_pairs = kv.reshape(B * C, JTOT, HID)

    QR = QBLK * CHUNK
    GRP = QBLK * JTOT // 2
    NGRP = PER_CORE * JTOT // GRP

    # partition-major packings: [.., p, ko, cols]
    wq_tt = np.asarray(Wq, dtype=f32).T  # [h, m]
    wq_t = np.ascontiguousarray(
        wq_tt.reshape(KSUB, P, MSUB, P).transpose(1, 2, 0, 3).astype(bf))
    wk_tt = np.asarray(Wk, dtype=f32).T  # [h, m]
    wk_t = np.ascontiguousarray(
        wk_tt.reshape(KSUB, P, MSUB, P).transpose(1, 2, 0, 3).astype(bf))
    wv_t = np.ascontiguousarray(
        np.asarray(Wv, dtype=f32).T.reshape(KSUB, P, HID)
        .transpose(1, 0, 2).astype(bf))
    bqk_t = np.ascontiguousarray(np.concatenate([
        np.asarray(bq, dtype=f32).reshape(MSUB, P).T,
        np.asarray(bk, dtype=f32).reshape(MSUB, P).T], axis=1))

    in_maps = []
    for ci in range(NCORES):
        sel = slice(ci * PER_CORE, (ci + 1) * PER_CORE)
        q_core = q_pairs[sel].reshape(PER_CORE * CHUNK, HID)
        kv_core = kv_pairs[sel].reshape(PER_CORE * JTOT, HID)
        q_tc = np.ascontiguousarray(
            q_core.T.reshape(KSUB, P, NBLK // 2, 2 * QR)
            .transpose(2, 1, 0, 3).astype(bf))
        kv_tc = np.ascontiguousarray(
            kv_core.T.reshape(KSUB, P, NGRP, GRP)
            .transpose(2, 1, 0, 3).astype(bf))
        in_maps.append({
            "q_t": q_tc,
            "kv_t": kv_tc,
            "wq_t": wq_t,
            "wk_t": wk_t,
            "wv_t": wv_t,
            "bqk_t": bqk_t,
        })
    return in_maps


def _unpack_output(results, bv):
    """results: list of 8 dicts with 'out' [16, 128, 512] -> full (B,S,HID)."""
    h = np.empty((NPAIRS, CHUNK, HID), dtype=np.float32)
    for ci in range(NCORES):
        arr = results[ci]["out"]
        a = arr.reshape(PER_CORE, 2, CHUNK, NHEADS // 2, HEAD)
        a = a.transpose(0, 2, 3, 1, 4).reshape(PER_CORE, CHUNK, HID)
        h[ci * PER_CORE:(ci + 1) * PER_CORE] = a
    h = h.reshape(B, C * CHUNK, HID)
    outp = np.zeros((B, S, HID), dtype=np.float32)
    # V bias deferred from the device (softmax weights sum to 1)
    outp[:, CHUNK - 1:] = h[:, : S - (CHUNK - 1)] \
        + np.asarray(bv, dtype=np.float32)
    return outp


def kernel(query, kv, Wq, bq, Wk, bk, Wv, bv):
    from concourse.bass_utils import run_bass_kernel_spmd

    if "nc" not in _CACHE:
        _CACHE["nc"] = _build()
    nc = _CACHE["nc"]

    in_maps = _prepare_inputs(query, kv, Wq, bq, Wk, bk, Wv, bv)
    res = run_bass_kernel_spmd(nc, in_maps, list(range(NCORES)))
    return _unpack_output(res.results, bv)
